# revision 5
# baseline (speedup 1.0000x reference)
"""Trainium2 Bass kernel for CombinedLoss (chamfer + density) on 8 NeuronCores.

Problem: B=4, N=M=8192, D=3.
  chamfer = mean_n min_m d2[b,n,m] + mean_m min_n d2[b,n,m],  d2 clamped >= 0
  density = mean |pred_densities|
  total   = chamfer_clipped + 0.1 * density

Strategy v2 (single-matrix pass; both chamfer directions from one d2 pass):
  - 8 cores = 4 batches x 2 row-halves. Core c handles batch c//2 and pred
    rows [h*4096, (h+1)*4096) vs ALL 8192 target points.
  - One K=24 matmul group per PSUM tile computes exact-f32 d2 (bf16
    triple-split operands, in-array f32 accumulation, single instruction
    per 512-col slab).
  - Per row tile (128 rows x 8192 cols = 4 PSUM groups of [128,2048] f32):
      * ScalarE evacuates each group to one fp16 SBUF tile cp[128,8192]
        (ScalarE has a PSUM port and nothing else to do).
      * VectorE: ONE tensor_tensor_reduce fuses min(cp_lo, cp_hi) with the
        free-axis min-reduce -> row-min of all 8192 cols in a single op.
      * VectorE: colacc[128,8192] fp16 gets one 2x-rate tensor_tensor min
        per tile (tile 0: a 4x-rate tensor_copy).
  - rowred[p, t] = min over all m for row t*128+p -> cham_x on host.
  - colacc[p, j] = min over rows {t*128+p} of d2[., j]; host finishes the
    128-way partition min and the cross-core (row-half) min -> cham_y.
  - Host: clamp mins at 0, means in f64, density term, assemble scalars.
"""

import os
from contextlib import ExitStack

import ml_dtypes
import numpy as np

import concourse.tile as tile
from concourse import bacc, mybir
from concourse.bass_utils import run_bass_kernel_spmd

B, N, M, D = 4, 8192, 8192, 3
R = N // 2          # rows per core
NT = R // 128       # 32 row tiles
NG = M // 2048      # 4 column groups
K = 24              # contraction rows of the distance matmul

BF16 = ml_dtypes.bfloat16

# mode flags:
#   "w"  1024-wide matmuls (2 PSUM banks per mm) instead of 512
MODE = os.environ.get("CHAMFER_MODE", "")


def _split3(a_f64):
    """Split values into 3 bf16 parts summing (near-)exactly to the input."""
    p0 = a_f64.astype(BF16)
    r1 = a_f64 - p0.astype(np.float64)
    p1 = r1.astype(BF16)
    r2 = r1 - p1.astype(np.float64)
    p2 = r2.astype(BF16)
    return p0, p1, p2


def _build_operands(rows_pts, cols_pts):
    """Stationary [K, R] and moving [K, ncols] bf16 matrices so that
    (stat.T @ mov)[i, j] = ||rows_pts[i] - cols_pts[j]||^2 in f32-grade accuracy.
    """
    a = rows_pts.astype(np.float64)
    b = cols_pts.astype(np.float64)
    a2 = (a * a).sum(-1)
    b2 = (b * b).sum(-1)
    ah, am, al = _split3(a.T)      # each [3, R]
    bh, bm, bl = _split3(b.T)      # each [3, ncols]
    a2h, a2m, a2l = _split3(a2)    # [R]
    b2h, b2m, b2l = _split3(b2)    # [ncols]

    nr, ncols = a.shape[0], b.shape[0]
    S = np.zeros((K, nr), BF16)
    Mv = np.zeros((K, ncols), BF16)
    ones_r = np.ones((nr,), BF16)
    ones_c = np.ones((ncols,), BF16)

    def neg2(t):
        return (-2.0 * t.astype(np.float32)).astype(BF16)  # exact for bf16 input

    # rows ordered largest magnitude first for benign accumulation order
    S[0], Mv[0] = a2h, ones_c
    S[1], Mv[1] = ones_r, b2h
    S[2:5], Mv[2:5] = neg2(ah), bh          # hh
    S[5], Mv[5] = a2m, ones_c
    S[6], Mv[6] = ones_r, b2m
    S[7:10], Mv[7:10] = neg2(ah), bm        # hm
    S[10:13], Mv[10:13] = neg2(am), bh      # mh
    S[13], Mv[13] = a2l, ones_c
    S[14], Mv[14] = ones_r, b2l
    S[15:18], Mv[15:18] = neg2(ah), bl      # hl
    S[18:21], Mv[18:21] = neg2(al), bh      # lh
    S[21:24], Mv[21:24] = neg2(am), bm      # mm
    return S, Mv


def _emit_pass(ctx, tc, pools, stat_ap, mov_ap, rowred_ap, colacc_ap, mode):
    nc = tc.nc
    persist, cps, junks, psum = pools
    f32 = mybir.dt.float32
    bf16 = mybir.dt.bfloat16
    fp16 = mybir.dt.float16
    MIN = mybir.AluOpType.min

    mov_sb = persist.tile([K, M], bf16, tag="mov")
    stat_sb = persist.tile([K, R], bf16, tag="stat")
    for c in range(4):
        nc.sync.dma_start(
            mov_sb[:, c * (M // 4) : (c + 1) * (M // 4)],
            mov_ap[:, c * (M // 4) : (c + 1) * (M // 4)],
        )
    for c in range(2):
        nc.sync.dma_start(
            stat_sb[:, c * (R // 2) : (c + 1) * (R // 2)],
            stat_ap[:, c * (R // 2) : (c + 1) * (R // 2)],
        )

    rowred = persist.tile([128, NT], f32, tag="rowred")
    colacc = persist.tile([128, M], fp16, tag="colacc")

    mm_w = 1024 if "w" in mode else 512
    n_mm = 2048 // mm_w

    for t in range(NT):
        cp = cps.tile([128, M], fp16, tag="cp")
        for g in range(NG):
            ps = psum.tile([128, 2048], f32, tag="ps")
            for s in range(n_mm):
                nc.tensor.matmul(
                    ps[:, mm_w * s : mm_w * (s + 1)],
                    lhsT=stat_sb[:, 128 * t : 128 * (t + 1)],
                    rhs=mov_sb[:, 2048 * g + mm_w * s : 2048 * g + mm_w * (s + 1)],
                    start=True,
                    stop=True,
                )
            nc.scalar.copy(cp[:, 2048 * g : 2048 * (g + 1)], ps[:])

        # row-min of all 8192 columns in ONE op: running-min scan over the
        # two 4096-wide halves; the last scan column is the row-min.
        # (tensor_tensor_reduce would also fuse this but faults at runtime
        # on this hardware path; the scan is HW-verified for min.)
        h = junks.tile([128, 4096], fp16, tag="h")
        nc.vector.tensor_tensor_scan(
            h[:], cp[:, 0:4096], cp[:, 4096:8192], 1.0e30, op0=MIN, op1=MIN
        )
        nc.vector.tensor_copy(rowred[:, t : t + 1], h[:, 4095:4096])

        # column-min accumulator update (2x-rate fp16 tensor_tensor); the
        # last tile updates in halves so the output DMA can start early.
        if t == 0:
            nc.vector.tensor_copy(colacc[:], cp[:])
        elif t == NT - 1:
            for half in range(2):
                sl = slice(4096 * half, 4096 * (half + 1))
                nc.vector.tensor_tensor(colacc[:, sl], colacc[:, sl], cp[:, sl], op=MIN)
                nc.sync.dma_start(colacc_ap[:, sl], colacc[:, sl])
        else:
            nc.vector.tensor_tensor(colacc[:], colacc[:], cp[:], op=MIN)

    nc.sync.dma_start(rowred_ap[:], rowred[:])


def _build_program(rep: int = 1, mode: str | None = None):
    mode = MODE if mode is None else mode
    nc = bacc.Bacc("TRN2", target_bir_lowering=False, debug=False, num_devices=8)
    bf16 = mybir.dt.bfloat16
    f32 = mybir.dt.float32
    fp16 = mybir.dt.float16
    stat = nc.dram_tensor("stat", [K, R], bf16, kind="ExternalInput").ap()
    mov = nc.dram_tensor("mov", [K, M], bf16, kind="ExternalInput").ap()
    rowred = nc.dram_tensor("rowred", [128, NT], f32, kind="ExternalOutput").ap()
    colacc = nc.dram_tensor("colacc", [128, M], fp16, kind="ExternalOutput").ap()

    with tile.TileContext(nc) as tc:
        with ExitStack() as ctx:
            persist = ctx.enter_context(tc.tile_pool(name="persist", bufs=1))
            cps = ctx.enter_context(tc.tile_pool(name="cps", bufs=3))
            junks = ctx.enter_context(tc.tile_pool(name="junks", bufs=2))
            psum = ctx.enter_context(tc.tile_pool(name="psum", bufs=2, space="PSUM"))
            pools = (persist, cps, junks, psum)

            def body(_i=None):
                _emit_pass(ctx, tc, pools, stat, mov, rowred, colacc, mode)

            if rep == 1:
                body()
            else:
                with tc.For_i(0, rep, 1) as i:
                    body(i)
    nc.compile()
    return nc


_NC_CACHE = None


def _get_program():
    global _NC_CACHE
    if _NC_CACHE is None:
        _NC_CACHE = _build_program()
    return _NC_CACHE


def _make_in_maps(pred_points, target_points):
    in_maps = []
    for c in range(8):
        b, h = divmod(c, 2)
        x_half = pred_points[b, h * R : (h + 1) * R]
        S, Mv = _build_operands(x_half, target_points[b])
        in_maps.append({"stat": S, "mov": Mv})
    return in_maps


def kernel(pred_points, target_points, pred_densities):
    pred_points = np.asarray(pred_points, np.float32)
    target_points = np.asarray(target_points, np.float32)
    pred_densities = np.asarray(pred_densities, np.float32)

    nc = _get_program()
    in_maps = _make_in_maps(pred_points, target_points)
    res = run_bass_kernel_spmd(nc, in_maps, core_ids=list(range(8)))

    mins_x = np.empty((B, N), np.float64)
    colmin = np.empty((B, M), np.float64)
    for c in range(8):
        b, h = divmod(c, 2)
        rr = np.asarray(res.results[c]["rowred"], np.float64)  # [128, NT]
        mins_x[b, h * R : (h + 1) * R] = rr.T.reshape(R)
        ca = np.asarray(res.results[c]["colacc"], np.float64).min(axis=0)  # [M]
        if h == 0:
            colmin[b] = ca
        else:
            colmin[b] = np.minimum(colmin[b], ca)

    cham_x = np.maximum(mins_x, 0.0).mean()
    cham_y = np.maximum(colmin, 0.0).mean()
    chamfer = np.clip(cham_x + cham_y, 0.0, 1.0e6)
    density = np.abs(pred_densities.astype(np.float64)).mean()
    total = 1.0 * chamfer + 0.1 * density
    return (
        np.float32(total),
        np.float32(chamfer),
        np.float32(density),
    )
